# revision 19
# baseline (speedup 1.0000x reference)
"""Distributed attention kernel for Trainium2 (8 NeuronCores).

Problem: B=2, L=2048, DIM=1024, H=16 heads, HD=64.
  qkv = x @ Wqkv; q,k = rmsnorm per head (+scales); RoPE(q, k);
  scores = q k^T / sqrt(HD); p = softmax(scores); o = p v;
  out = o @ Wproj + bproj.

Sharding: tensor-parallel over heads -- 2 heads per core.

Key perf idea vs the previous version: keep the PE array in ONE tiling
configuration -- every matmul is full (128,128) mode -- so the tensor
engine streams back-to-back and the HAM clock gate stays at 8/8
(2.4 GHz).  Half-contractions (per-head scores, K=64) and small-M
matmuls (o-proj with the softmax-denominator ones column, M=65) are
zero-PADDED to 128 instead of sliced: same cycle count, full array
activity, no tiling-mode switches/drains.

  - scores: lhsT = kTnA (rows 64:128 zero) / kTnB (rows 0:64 zero),
    rhs = qTn (both heads stacked) -> per-head scores in one
    (128,128)-mode matmul each.
  - o: lhsT = vA tile [128m, 128] = [vA feats | ones | 0...];
    vB tile = [0... | ones | vB feats] so head B's output lands in
    PSUM partitions 64:128 directly.
  - exp: one ACT instruction per (m, s) over a [128, 1024] PSUM tile
    spanning both heads' score banks (amortizes the ACT init latency).
  - rmsnorm reductions/broadcasts + softmax-denominator broadcast are
    K- or M-padded to 128 with zero indicator rows/cols.
  - l-tiles are interleaved across the 4 chunks of each batch so the
    head->sequence AllToAll splits into 4 uniform 256 KB calls, each
    overlapped behind the next attention block; only the last is
    exposed.
"""

import sys

if "/opt/trn_rl_repo" not in sys.path:
    sys.path.insert(0, "/opt/trn_rl_repo")

import numpy as np
import ml_dtypes

B, L, DIM, H, HD = 2, 2048, 1024, 16, 64
NC = 8
HPC = H // NC          # heads per core = 2
BL = B * L             # 4096 flattened rows
CH = 512               # l-chunk size
NCH = BL // CH         # 8 chunks
EPS = 1e-6
THETA = 10000.0
F = 3 * HPC * HD       # 384 qkv features per core

BF = ml_dtypes.bfloat16
_CACHE = {}


def _rope_tables():
    inv_freq = 1.0 / (THETA ** (np.arange(0, HD, 2, dtype=np.float64) / HD))
    ang = np.arange(L, dtype=np.float64)[None, :] * inv_freq[:, None]  # [32,L]
    return np.cos(ang), np.sin(ang)


def _make_tables(scale, fold):
    """[64, L] bf16 cos/sin coefficient tables, per-feature scale folded in.

    Device computes, per head (rows r0..r0+63 of the qkv tile):
      tc = src[0:64] * ct
      ts[0:32]  = src[32:64] * st[32:64]   (pre-swapped, sign folded)
      ts[32:64] = src[0:32]  * st[0:32]
      out = tc + ts
    which equals rotate-half RoPE with scale/fold applied.
    """
    c, s = _rope_tables()
    ct = np.empty((HD, L), np.float64)
    st = np.empty((HD, L), np.float64)
    ct[0:32] = c * (scale[0:32, None] * fold)
    ct[32:64] = c * (scale[32:64, None] * fold)
    st[0:32] = s * (scale[0:32, None] * fold)
    st[32:64] = -s * (scale[32:64, None] * fold)
    return ct.astype(BF), st.astype(BF)


def _host_inputs(x, Wqkv, q_scale, k_scale, Wproj, bproj):
    x2 = np.ascontiguousarray(np.asarray(x, np.float32).reshape(BL, DIM))
    xT = np.ascontiguousarray(x2.T.astype(BF))              # [DIM, BL] bf16
    Wqkv = np.asarray(Wqkv, np.float32)
    Wq = Wqkv[:, 0 * DIM:1 * DIM].reshape(DIM, H, HD)
    Wk = Wqkv[:, 1 * DIM:2 * DIM].reshape(DIM, H, HD)
    Wv = Wqkv[:, 2 * DIM:3 * DIM].reshape(DIM, H, HD)

    qc, qs = _make_tables(np.asarray(q_scale, np.float64), 1.0 / np.sqrt(HD))
    kc, ks = _make_tables(np.asarray(k_scale, np.float64), 1.0)
    qc = np.concatenate([qc, qc], 0)   # [128, L]: same table for both heads
    qs = np.concatenate([qs, qs], 0)
    kc = np.concatenate([kc, kc], 0)
    ks = np.concatenate([ks, ks], 0)

    # ssq indicator: out[j] = sum_k sc_ind[k, j] * sq[k]; col0 = head A sum,
    # col1 = head B sum, cols 2:128 zero (M padded to 128).
    sc_ind = np.zeros((128, 128), BF)
    sc_ind[0:64, 0] = 1.0
    sc_ind[64:128, 1] = 1.0
    # inv-rms broadcast: row0 -> partitions 0:64, row1 -> 64:128, with the
    # 8 = sqrt(HD) mean-square fold; rows 2:128 zero (K padded to 128).
    bc_ind = np.zeros((128, 128), BF)
    bc_ind[0, 0:64] = 8.0
    bc_ind[1, 64:128] = 8.0
    # softmax denominator broadcast: row0 (1/dA) -> partitions 0:64,
    # row1 (1/dB) -> partitions 64:128.
    rb_ind = np.zeros((128, 128), BF)
    rb_ind[0, 0:64] = 1.0
    rb_ind[32, 64:128] = 1.0
    ident = np.eye(128, dtype=BF)
    wp = np.ascontiguousarray(np.asarray(Wproj, np.float32).astype(BF))
    bp = np.ascontiguousarray(
        np.asarray(bproj, np.float32).reshape(8, 128).T)    # [128, 8]

    shared = dict(xT=xT, qc=qc, qs=qs, kc=kc, ks=ks, sc_ind=sc_ind,
                  bc_ind=bc_ind, rb_ind=rb_ind, ident=ident, wp=wp, bp=bp)
    in_maps = []
    for c in range(NC):
        hA, hB = HPC * c, HPC * c + 1
        wqc = np.concatenate(
            [Wq[:, hA], Wq[:, hB], Wk[:, hA], Wk[:, hB], Wv[:, hA], Wv[:, hB]],
            axis=1)                                        # [DIM, 384]
        m = dict(shared)
        m["wq"] = np.ascontiguousarray(wqc.astype(BF))
        in_maps.append(m)
    return in_maps


def _build():
    import concourse.bass as bass  # noqa: F401
    import concourse.mybir as mybir
    import concourse.tile as tile
    from concourse import bacc

    fp32 = mybir.dt.float32
    bf16 = mybir.dt.bfloat16
    AF = mybir.ActivationFunctionType

    nc = bacc.Bacc("TRN2", target_bir_lowering=False, debug=False,
                   num_devices=NC)

    xT = nc.dram_tensor("xT", [DIM, BL], bf16, kind="ExternalInput")
    wq = nc.dram_tensor("wq", [DIM, F], bf16, kind="ExternalInput")
    qc = nc.dram_tensor("qc", [128, L], bf16, kind="ExternalInput")
    qs = nc.dram_tensor("qs", [128, L], bf16, kind="ExternalInput")
    kc = nc.dram_tensor("kc", [128, L], bf16, kind="ExternalInput")
    ks = nc.dram_tensor("ks", [128, L], bf16, kind="ExternalInput")
    sc_ind_d = nc.dram_tensor("sc_ind", [128, 128], bf16,
                              kind="ExternalInput")
    bc_ind_d = nc.dram_tensor("bc_ind", [128, 128], bf16,
                              kind="ExternalInput")
    rb_ind_d = nc.dram_tensor("rb_ind", [128, 128], bf16,
                              kind="ExternalInput")
    ident_d = nc.dram_tensor("ident", [128, 128], bf16, kind="ExternalInput")
    wp_d = nc.dram_tensor("wp", [DIM, DIM], bf16, kind="ExternalInput")
    bp_d = nc.dram_tensor("bp", [128, 8], fp32, kind="ExternalInput")
    out_d = nc.dram_tensor("out", [DIM, CH], fp32, kind="ExternalOutput")

    with tile.TileContext(nc) as tc:
        with (
            tc.tile_pool(name="consts", bufs=1) as consts,
            tc.tile_pool(name="wqp", bufs=1) as wqp,
            tc.tile_pool(name="tabs", bufs=1) as tabs,
            tc.tile_pool(name="acts", bufs=1) as acts,
            tc.tile_pool(name="wppool", bufs=1) as wppool,
            tc.tile_pool(name="dram", bufs=1, space="DRAM") as dram,
        ):
            sc_ind = consts.tile([128, 128], bf16)
            nc.sync.dma_start(sc_ind[:], sc_ind_d[:])
            bc_ind = consts.tile([128, 128], bf16)
            nc.sync.dma_start(bc_ind[:], bc_ind_d[:])
            rb_ind = consts.tile([128, 128], bf16)
            nc.sync.dma_start(rb_ind[:], rb_ind_d[:])
            ident = consts.tile([128, 128], bf16)
            nc.sync.dma_start(ident[:], ident_d[:])
            bp_sb = consts.tile([128, 8], fp32)
            nc.sync.dma_start(bp_sb[:], bp_d[:])

            wq_sb = []
            for kk in range(8):
                t = wqp.tile([128, F], bf16, name=f"wq{kk}")
                nc.sync.dma_start(t[:], wq[128 * kk:128 * (kk + 1), :])
                wq_sb.append(t)

            qc_sb = tabs.tile([128, L], bf16)
            nc.sync.dma_start(qc_sb[:], qc[:])
            qs_sb = tabs.tile([128, L], bf16)
            nc.sync.dma_start(qs_sb[:], qs[:])
            kc_sb = tabs.tile([128, L], bf16)
            nc.sync.dma_start(kc_sb[:], kc[:])
            ks_sb = tabs.tile([128, L], bf16)
            nc.sync.dma_start(ks_sb[:], ks[:])


            # persistent per-batch activations
            qTn = [acts.tile([128, L], bf16, name=f"qTn{b}") for b in range(B)]
            kTnA = [acts.tile([128, L], bf16, name=f"kTnA{b}")
                    for b in range(B)]
            kTnB = [acts.tile([128, L], bf16, name=f"kTnB{b}")
                    for b in range(B)]
            # v per (b, head): m-tile-major blocks of 128 cols:
            #   vA block: [64 feats | ones | 0*63]; vB block: [0*63 | ones | 64 feats]
            vA = [acts.tile([128, 16 * 128], bf16, name=f"vA{b}")
                  for b in range(B)]
            vB = [acts.tile([128, 16 * 128], bf16, name=f"vB{b}")
                  for b in range(B)]
            # inv-rms staging (rows 0:2 live, rest zero), cols by chunk-in-b
            ivq = [acts.tile([128, 4 * CH], bf16, name=f"ivq{b}")
                   for b in range(B)]
            ivk = [acts.tile([128, 4 * CH], bf16, name=f"ivk{b}")
                   for b in range(B)]
            # softmax denominator recips (rows 0:2 live, rest zero)
            rcb = acts.tile([128, CH], bf16, name="rcb")

            for b in range(B):
                nc.gpsimd.memset(kTnA[b][64:128, :], 0.0)
                nc.gpsimd.memset(kTnB[b][0:64, :], 0.0)
                nc.gpsimd.memset(vA[b][:], 0.0)
                nc.gpsimd.memset(vB[b][:], 0.0)
                nc.gpsimd.memset(ivq[b][:], 0.0)
                nc.gpsimd.memset(ivk[b][:], 0.0)
                for mt in range(16):
                    nc.gpsimd.memset(vA[b][:, 128 * mt + 64:128 * mt + 65],
                                     1.0)
                    nc.gpsimd.memset(vB[b][:, 128 * mt + 32:128 * mt + 33],
                                     1.0)
            nc.gpsimd.memset(rcb[:], 0.0)

            # l-tiles of widths {128,128,128,64,64}; A2A calls carry
            # {t0+t1, t2, t3, t4} so the exposed final call is only 64
            # cols (128 KB) and the final proj round is tiny.
            a2a_in = [dram.tile([NC * 128, 256], bf16, name="a2a_in0"),
                      dram.tile([NC * 128, 128], bf16, name="a2a_in1"),
                      dram.tile([NC * 128, 64], bf16, name="a2a_in2"),
                      dram.tile([NC * 128, 64], bf16, name="a2a_in3")]
            a2a_out = [dram.tile([NC * 128, 256], bf16, name="a2a_out0"),
                       dram.tile([NC * 128, 128], bf16, name="a2a_out1"),
                       dram.tile([NC * 128, 64], bf16, name="a2a_out2"),
                       dram.tile([NC * 128, 64], bf16, name="a2a_out3")]
            # (qoff within chunk, width, a2a buffer index, col offset)
            LT = [(0, 128, 0, 0), (128, 128, 0, 128), (256, 128, 1, 0),
                  (384, 64, 2, 0), (448, 64, 3, 0)]

            # ---------- phase 1: qkv + rmsnorm + rope + v transpose -------
            # Pipelined emission: chunk ch's qkv matmuls + PSUM->SBUF
            # evacuation are emitted first; the dependent tail (rmsnorm
            # broadcasts, rope, v transposes) is deferred until after the
            # NEXT chunk's dense matmul stream so the PE queue never stalls
            # behind ACT/DVE chains (head-of-line blocking).
            with (
                tc.tile_pool(name="xt", bufs=8) as xtp,
                tc.tile_pool(name="ps", bufs=4, space="PSUM") as ps,
                tc.tile_pool(name="pred", bufs=1, space="PSUM") as pred,
                tc.tile_pool(name="pbc", bufs=2, space="PSUM") as pbc,
                tc.tile_pool(name="ptr", bufs=1, space="PSUM") as ptr,
                tc.tile_pool(name="sqp", bufs=4) as sqp,
                tc.tile_pool(name="sdp", bufs=6) as sdp,
                tc.tile_pool(name="tmp", bufs=8) as tmpp,
                tc.tile_pool(name="vt", bufs=2) as vtp,
            ):
                staged = {}
                xt_pair = {}

                def load_xt_pair(pr):
                    # one [128, 1024] DMA per k-tile covers chunks 2pr, 2pr+1
                    c0 = 2 * CH * pr
                    tiles = []
                    for kk in range(8):
                        t = xtp.tile([128, 2 * CH], bf16, tag="xt")
                        nc.sync.dma_start(
                            t[:], xT[128 * kk:128 * (kk + 1), c0:c0 + 2 * CH])
                        tiles.append(t)
                    xt_pair[pr] = tiles

                def emit_head(ch):
                    b, cc = ch // 4, ch % 4
                    half = slice(CH * (ch % 2), CH * (ch % 2) + CH)
                    xt = [t[:, half] for t in xt_pair[ch // 2]]
                    pst = []
                    for tix in range(3):
                        p = ps.tile([128, CH], fp32, tag="ps")
                        for kk in range(8):
                            nc.tensor.matmul(
                                p[:], wq_sb[kk][:, 128 * tix:128 * (tix + 1)],
                                xt[kk], start=(kk == 0), stop=(kk == 7))
                        pst.append(p)
                    # evacuate qkv PSUM to SBUF staging (ACT only)
                    sqs, stgs = [], []
                    for tix in range(2):
                        sq = sqp.tile([128, CH], bf16, tag="sq")
                        nc.scalar.activation(sq[:], pst[tix][:], AF.Square)
                        sqs.append(sq)
                        stg = tmpp.tile([128, CH], bf16, tag="stg")
                        nc.scalar.activation(stg[:], pst[tix][:], AF.Copy)
                        stgs.append(stg)
                    vtt = vtp.tile([128, CH], bf16, tag="vt")
                    nc.scalar.activation(vtt[:], pst[2][:], AF.Copy)
                    staged[ch] = (sqs, stgs, vtt)

                def emit_tail(ch):
                    b, cc = ch // 4, ch % 4
                    lsl = slice(CH * cc, CH * cc + CH)
                    sqs, stgs, vtt = staged.pop(ch)
                    for tix, ivt in ((0, ivq[b]), (1, ivk[b])):
                        ssq = pred.tile([128, CH], fp32, tag="ssq")
                        nc.tensor.matmul(ssq[:], sc_ind[:], sqs[tix][:],
                                         start=True, stop=True)
                        sd = sdp.tile([2, CH], fp32, tag="sd")
                        nc.scalar.activation(sd[:], ssq[0:2, :], AF.Sqrt)
                        iv = sdp.tile([2, CH], fp32, tag="iv")
                        nc.vector.reciprocal_approx_fast(iv[:], sd[:])
                        nc.vector.tensor_copy(ivt[0:2, lsl], iv[:])
                    invbq = pbc.tile([128, CH], fp32, tag="invb")
                    nc.tensor.matmul(invbq[:], bc_ind[:], ivq[b][:, lsl],
                                     start=True, stop=True)
                    invbk = pbc.tile([128, CH], fp32, tag="invb")
                    nc.tensor.matmul(invbk[:], bc_ind[:], ivk[b][:, lsl],
                                     start=True, stop=True)
                    for tix, ct, stb, invb in (
                            (0, qc_sb, qs_sb, invbq),
                            (1, kc_sb, ks_sb, invbk)):
                        stg = stgs[tix]
                        tc_ = tmpp.tile([128, CH], bf16, tag="tc")
                        nc.vector.tensor_mul(tc_[:], stg[:], ct[:, lsl])
                        ts_ = tmpp.tile([128, CH], bf16, tag="ts")
                        eng = nc.vector if tix == 0 else nc.gpsimd
                        for r0 in (0, 64):
                            eng.tensor_mul(
                                ts_[r0:r0 + 32, :], stg[r0 + 32:r0 + 64, :],
                                stb[r0 + 32:r0 + 64, lsl])
                            eng.tensor_mul(
                                ts_[r0 + 32:r0 + 64, :], stg[r0:r0 + 32, :],
                                stb[r0:r0 + 32, lsl])
                        o12 = tmpp.tile([128, CH], bf16, tag="o12")
                        nc.vector.tensor_add(o12[:], tc_[:], ts_[:])
                        if tix == 0:
                            nc.vector.tensor_mul(qTn[b][:, lsl], invb[:],
                                                 o12[:])
                        else:
                            nc.vector.tensor_mul(kTnA[b][0:64, lsl],
                                                 invb[0:64, :], o12[0:64, :])
                            nc.vector.tensor_mul(kTnB[b][64:128, lsl],
                                                 invb[64:128, :],
                                                 o12[64:128, :])
                    tp = ptr.tile([128, CH], bf16, tag="tp")
                    for blk in range(4):
                        nc.tensor.transpose(
                            tp[:, 128 * blk:128 * (blk + 1)],
                            vtt[:, 128 * blk:128 * (blk + 1)], ident[:])
                    tp3 = tp[:].rearrange("p (blk c) -> p blk c", blk=4)
                    vA3 = vA[b][:].rearrange("p (mt c) -> p mt c", mt=16)
                    vB3 = vB[b][:].rearrange("p (mt c) -> p mt c", mt=16)
                    nc.vector.tensor_copy(
                        vA3[:, 4 * cc:4 * cc + 4, 0:64], tp3[:, :, 0:64])
                    nc.vector.tensor_copy(
                        vB3[:, 4 * cc:4 * cc + 4, 64:128], tp3[:, :, 64:128])

                load_xt_pair(0)
                load_xt_pair(1)
                for ch in range(NCH):
                    if ch % 2 == 0 and ch // 2 + 2 < 4:
                        load_xt_pair(ch // 2 + 2)
                    emit_head(ch)
                    if ch > 0:
                        emit_tail(ch - 1)
                emit_tail(NCH - 1)

            # ---------- wproj load (overlaps attention) ----------
            wp_sb = []
            for ff in range(8):
                t = wppool.tile([128, DIM], bf16, name=f"wp{ff}")
                nc.sync.dma_start(t[:], wp_d[128 * ff:128 * (ff + 1), :])
                wp_sb.append(t)

            # ---------- phase 2: attention ----------
            # l-tile s of batch b = cols {512*c + 128*s : +128, c in 0..3}
            # of qTn[b] -> output col-block c goes to dest core 4b + c.
            with (
                tc.tile_pool(name="stp", bufs=2, space="PSUM") as stp,
                tc.tile_pool(name="pop", bufs=4, space="PSUM") as pop,
                tc.tile_pool(name="ptp", bufs=3) as ptp,
                tc.tile_pool(name="rcp", bufs=2) as rcp,
                tc.tile_pool(name="otp", bufs=2) as otp,
            ):
                def emit_mloop(t, b):
                    qoff, w, _, _ = LT[t]
                    N = 4 * w
                    qsl = qTn[b][:].rearrange(
                        "p (c s) -> p c s", c=4)[:, :, qoff:qoff + w]
                    poA = pop.tile([128, N], fp32, tag="po",
                                   name=f"poA{t}{b}")
                    poB = pop.tile([128, N], fp32, tag="po",
                                   name=f"poB{t}{b}")
                    for m in range(16):
                        mo = 128 * m
                        st = stp.tile([128, 2 * N], fp32, tag="st")
                        nc.tensor.matmul(
                            st[:, 0:N], kTnA[b][:, mo:mo + 128], qsl,
                            start=True, stop=True)
                        nc.tensor.matmul(
                            st[:, N:2 * N], kTnB[b][:, mo:mo + 128],
                            qsl, start=True, stop=True)
                        pt = ptp.tile([128, 2 * N], bf16, tag="pt")
                        nc.scalar.activation(pt[:], st[:], AF.Exp)
                        nc.tensor.matmul(
                            poA[:], vA[b][:, mo:mo + 128], pt[:, 0:N],
                            start=(m == 0), stop=(m == 15))
                        nc.tensor.matmul(
                            poB[:], vB[b][:, mo:mo + 128],
                            pt[:, N:2 * N],
                            start=(m == 0), stop=(m == 15))
                    return poA, poB

                def emit_norm(t, b, poA, poB):
                    qoff, w, bi, so = LT[t]
                    N = 4 * w
                    # denominators: dA = poA[64], dB = poB[32]
                    rcA = rcp.tile([1, N], fp32, tag="rcA")
                    nc.vector.tensor_copy(rcA[:], poA[64:65, :])
                    rcB = rcp.tile([1, N], fp32, tag="rcB")
                    nc.vector.tensor_copy(rcB[:], poB[32:33, :])
                    rvA = rcp.tile([1, N], fp32, tag="rvA")
                    nc.vector.reciprocal_approx_fast(rvA[:], rcA[:])
                    rvB = rcp.tile([1, N], fp32, tag="rvB")
                    nc.vector.reciprocal_approx_fast(rvB[:], rcB[:])
                    nc.vector.tensor_copy(rcb[0:1, 0:N], rvA[:])
                    nc.vector.tensor_copy(rcb[32:33, 0:N], rvB[:])
                    invd = stp.tile([128, N], fp32, tag="st",
                                    name=f"invd{t}{b}")
                    nc.tensor.matmul(invd[:], rb_ind[:], rcb[:, 0:N],
                                     start=True, stop=True)
                    invd_s = rcp.tile([128, N], bf16, tag="invd_s")
                    nc.vector.tensor_copy(invd_s[:], invd[:])
                    ot = otp.tile([128, N], bf16, tag="ot")
                    nc.vector.tensor_mul(ot[0:64, :], poA[0:64, :],
                                         invd_s[0:64, :])
                    nc.vector.tensor_mul(ot[64:128, :], poB[64:128, :],
                                         invd_s[64:128, :])
                    dst = a2a_in[bi][:].rearrange(
                        "(j p) i -> p j i", p=128)[:, 4 * b:4 * b + 4,
                                                   so:so + w]
                    srcv = ot[:].rearrange("p (c i) -> p c i", c=4)
                    nc.sync.dma_start(dst, srcv)

                # software-pipelined: block k's normalize tail is emitted
                # after block k+1's m-loop so the PE queue never stalls on
                # the DVE reciprocal chain; the A2A for buffer bi fires
                # once its last contributing block is normalized.
                blocks = [(t, b) for t in range(len(LT)) for b in range(B)]
                pending = None
                for t, b in blocks:
                    poA, poB = emit_mloop(t, b)
                    if pending is not None:
                        pt_, pb_, pA_, pB_ = pending
                        emit_norm(pt_, pb_, pA_, pB_)
                        if pb_ == B - 1 and pt_ >= 1:
                            ci = max(0, pt_ - 1)
                            nc.gpsimd.collective_compute(
                                "AllToAll", mybir.AluOpType.bypass,
                                replica_groups=[list(range(NC))],
                                ins=[a2a_in[ci][:]],
                                outs=[a2a_out[ci][:]],
                            )
                    pending = (t, b, poA, poB)
                t, b, poA, poB = pending
                emit_norm(t, b, poA, poB)
                nc.gpsimd.collective_compute(
                    "AllToAll", mybir.AluOpType.bypass,
                    replica_groups=[list(range(NC))],
                    ins=[a2a_in[3][:]],
                    outs=[a2a_out[3][:]],
                )

            # ---------- phase 3: output projection ----------
            with (
                tc.tile_pool(name="ofp", bufs=8) as ofp,
                tc.tile_pool(name="prp", bufs=8, space="PSUM") as prp,
                tc.tile_pool(name="obp", bufs=4) as obp,
            ):
                of = []
                for ff in range(8):
                    t = ofp.tile([128, CH], bf16, tag="of", name=f"of{ff}")
                    of.append(t)
                pr = []
                for dd in range(8):
                    pr.append(prp.tile([128, CH], fp32, tag="pr",
                                       name=f"pr{dd}"))
                for rnd, (lo, hi) in enumerate(
                        ((0, 256), (256, 384), (384, 448), (448, 512))):
                    for ff in range(8):
                        nc.sync.dma_start(
                            of[ff][:, lo:hi],
                            a2a_out[rnd][128 * ff:128 * (ff + 1), :])
                    for dd in range(8):
                        for ff in range(8):
                            nc.tensor.matmul(
                                pr[dd][:, lo:hi],
                                wp_sb[ff][:, 128 * dd:128 * (dd + 1)],
                                of[ff][:, lo:hi],
                                start=(ff == 0), stop=(ff == 7))
                        ob = obp.tile([128, hi - lo], fp32, tag=f"ob{rnd}")
                        nc.vector.tensor_scalar_add(ob[:], pr[dd][:, lo:hi],
                                                    bp_sb[:, dd:dd + 1])
                        nc.sync.dma_start(
                            out_d[128 * dd:128 * (dd + 1), lo:hi], ob[:])

    nc.compile()
    return nc


def _run(inputs, trace=False, trace_kwargs=None):
    from concourse.bass_utils import run_bass_kernel_spmd

    if "nc" not in _CACHE:
        _CACHE["nc"] = _build()
    nc = _CACHE["nc"]
    in_maps = _host_inputs(**inputs)
    res = run_bass_kernel_spmd(
        nc, in_maps, core_ids=list(range(NC)), trace=trace,
        **(trace_kwargs or {}))
    return res


def kernel(x, Wqkv, q_scale, k_scale, Wproj, bproj):
    res = _run(dict(x=x, Wqkv=Wqkv, q_scale=q_scale, k_scale=k_scale,
                    Wproj=Wproj, bproj=bproj))
    outT = np.concatenate([res.results[c]["out"] for c in range(NC)], axis=1)
    return np.ascontiguousarray(outT.T).reshape(B, L, DIM).astype(np.float32)


if __name__ == "__main__":
    rng = np.random.default_rng(0)
    x = rng.standard_normal((B, L, DIM), dtype=np.float32)
    Wqkv_ = rng.standard_normal((DIM, 3 * DIM), dtype=np.float32) * DIM ** -0.5
    Wproj_ = rng.standard_normal((DIM, DIM), dtype=np.float32) * DIM ** -0.5
    out = kernel(x=x, Wqkv=Wqkv_, q_scale=np.ones(HD, np.float32),
                 k_scale=np.ones(HD, np.float32), Wproj=Wproj_,
                 bproj=np.zeros(DIM, np.float32))
    print(out.shape, out.dtype)
